# revision 105
# baseline (speedup 1.0000x reference)
"""Trainium2 Bass kernel for LocalAttnLayer (sliding-window attention block).

Sharding: 8 cores = (batch b in 0..3) x (sequence half s in 0..1).
Each core processes 2048 tokens; the 128-token look-backward halo is handled
with a ring of per-window K/V tiles (the previous chunk's last window tile
stays alive in the ring, so halo K/V is never recomputed or copied).

Per-core schedule (4 chunks x 512 tokens, window = 128):
  P(n): QKV projections from the x^T slab (QKV weights streamed in
        1KB-line blocks; W1/W2 are resident in SBUF, loaded once on the
        sync queue behind chunk 0's projection weights).
  A(n): per window, scores in 4-head PSUM groups.  Heads are permuted into
        parity slots (even heads use lhsT partitions 0:64, odd 64:128) so
        each PSUM tile/buffer only ever sees ONE lhsT partition offset —
        mixing offsets within a PSUM tile is rejected by the hardware.
        One Exp per group (bias -ln2(1e3) keeps f16 range), multiplicative
        causal mask on the DVE, AV matmuls with a ones-column appended to V
        giving the softmax denominator, then a fused
        (av * 1/den + (x+bv)) scalar_tensor_tensor epilogue.
        LN1/LN2 rstd = exp(-0.5*ln(var+eps)) and the table chooser is
        pinned to natural_log_exp_and_others, so the whole program needs
        exactly one LoadActFuncSet.  x-hat is transposed for the FF with a
        DMA transpose (XBAR), not PE matmuls.
  F(n): window-granular FF1 (per window pair) and FF2+LN2+store (per
        128-token tile).  The previous chunk's FF1(pair1) halves and
        FF2(t2)/FF2(t3) are emitted into the P->A handoff and the exp-wait
        bubbles of windows 0-1, and FF1(pair0)/FF2(t0,t1) fill windows 2-3,
        so the PE stream never head-of-line blocks on the softmax chain.

Host-side folds: attention scale+q_bias into Wq/bq; k_bias dropped (softmax
row-shift invariant); v_bias into the residual stream xp; ln1_g folded into
W1 rows (the transpose reads raw x-hat, keeping g1 off the critical path);
ln1_b folded into ff1/ff2 biases.  Outputs are stored f16 and cast to f32
on the host (gate is 2e-2; measured rel err ~1e-3).
"""

import os
import sys

for _p in ("/opt/trn_rl_repo", "/root/.axon_site/_ro/trn_rl_repo"):
    if os.path.isdir(_p) and _p not in sys.path:
        sys.path.insert(0, _p)

import numpy as np

# Model dims (hardcoded per the problem spec)
B, S, D = 4, 4096, 1024
H, DH = 16, 64
WIN = 128
FH = 2048
LN_EPS = 1e-5

# Per-core sharding
T = 2048          # own tokens per core
HALO = 128
NCH = 4           # chunks per core
CT = 512          # tokens per chunk
CW = CT // WIN    # windows per chunk = 4
NW = NCH * CW     # windows per core = 16

EBT = -6.931471805599453  # exp bias: scores are within +-15, so
                          # exp(s + EBT) stays well inside f16 range

_PROGRAM_CACHE = {}


def _build_program():
    import concourse.bass as bass
    import concourse.tile as tile
    from concourse import bacc, mybir
    from contextlib import ExitStack

    f16 = mybir.dt.float16
    f32 = mybir.dt.float32
    AF = mybir.ActivationFunctionType
    ALU = mybir.AluOpType

    # Pin the activation-table chooser to natural_log_exp_and_others (it
    # covers every ACT function this kernel uses: exp, ln, identity, copy,
    # relu), so the program needs exactly one LoadActFuncSet instead of
    # thrashing between the exp- and ln-sets every window.  Other sets are
    # emptied rather than removed so act_func_set ids keep their positions.
    _orig_gat = bacc.get_activation_tables

    def _pinned_gat(arch, _orig=_orig_gat):
        keep = "natural_log_exp_and_others"
        full = dict(_orig(arch))
        if keep in full:
            return {k: (v if k == keep else set()) for k, v in full.items()}
        return full

    nc = bacc.Bacc("TRN2", target_bir_lowering=False, debug=False, num_devices=8)

    # ---- DRAM tensors ----
    xT = nc.dram_tensor("xT", [D, HALO + T], f16, kind="ExternalInput").ap()
    xp = nc.dram_tensor("xp", [T, D], f16, kind="ExternalInput").ap()
    wq = nc.dram_tensor("wq", [D, D], f16, kind="ExternalInput").ap()
    wk = nc.dram_tensor("wk", [D, D], f16, kind="ExternalInput").ap()
    wv = nc.dram_tensor("wv", [D, D], f16, kind="ExternalInput").ap()
    bqd = nc.dram_tensor("bq", [128, 8], f32, kind="ExternalInput").ap()
    w1 = nc.dram_tensor("w1", [D, FH], f16, kind="ExternalInput").ap()
    b1d = nc.dram_tensor("b1", [128, 16], f32, kind="ExternalInput").ap()
    w2 = nc.dram_tensor("w2", [FH, D], f16, kind="ExternalInput").ap()
    b2d = nc.dram_tensor("b2", [1, D], f16, kind="ExternalInput").ap()
    g1d = nc.dram_tensor("g1", [D], f16, kind="ExternalInput").ap()
    g2d = nc.dram_tensor("g2", [D], f16, kind="ExternalInput").ap()
    bt2d = nc.dram_tensor("bt2", [D], f16, kind="ExternalInput").ap()
    cmkd = nc.dram_tensor("cmk", [WIN, WIN], f16, kind="ExternalInput").ap()
    m0d = nc.dram_tensor("m0", [WIN, WIN], f16, kind="ExternalInput").ap()
    outd = nc.dram_tensor("out", [T, D], f16, kind="ExternalOutput").ap()

    # DRAM views with the 128-partition dim first.  Feature index d maps to
    # (p, dt) as d = dt*128 + p (matches dma_start_transpose tile order).
    xT_r = xT.rearrange("(dt p) c -> p dt c", p=128)
    wq_r = wq.rearrange("(dt p) n -> p dt n", p=128)
    wk_r = wk.rearrange("(dt p) n -> p dt n", p=128)
    wv_r = wv.rearrange("(dt p) n -> p dt n", p=128)
    w1_r = w1.rearrange("(dt p) n -> p dt n", p=128)
    w2_r = w2.rearrange("(ht p) n -> p ht n", p=128)
    xp_r = xp.rearrange("(n p) d -> n p d", p=128)
    out_r = outd.rearrange("(n p) d -> n p d", p=128)

    def bcast_ap(src_ap, parts=128):
        return bass.AP(
            tensor=src_ap.tensor,
            offset=src_ap.offset,
            ap=[[0, parts]] + [list(x) for x in src_ap.ap],
        )

    with tile.TileContext(nc) as tc, ExitStack() as ctx:
        # ---- pools ----
        singles = ctx.enter_context(tc.tile_pool(name="singles", bufs=1))
        w12_pool = ctx.enter_context(tc.tile_pool(name="w12", bufs=1))
        wblk_pool = ctx.enter_context(tc.tile_pool(name="wblk", bufs=3))
        xt_pool = ctx.enter_context(tc.tile_pool(name="xt", bufs=2))
        qt_pool = ctx.enter_context(tc.tile_pool(name="qt", bufs=2))
        kt_pool = ctx.enter_context(tc.tile_pool(name="kt", bufs=5))
        v_pool = ctx.enter_context(tc.tile_pool(name="v", bufs=5))
        es_pool = ctx.enter_context(tc.tile_pool(name="es", bufs=2))
        xp_pool = ctx.enter_context(tc.tile_pool(name="xpp", bufs=2))
        at_pool = ctx.enter_context(tc.tile_pool(name="at", bufs=2))
        tmp_pool = ctx.enter_context(tc.tile_pool(name="tmp", bufs=1))
        xhb_pool = ctx.enter_context(tc.tile_pool(name="xhb", bufs=6))
        xht_pool = ctx.enter_context(tc.tile_pool(name="xht", bufs=2))
        ht_pool = ctx.enter_context(tc.tile_pool(name="ht", bufs=2))
        p2_pool = ctx.enter_context(tc.tile_pool(name="p2", bufs=2))
        oh_pool = ctx.enter_context(tc.tile_pool(name="oh", bufs=2))
        small = ctx.enter_context(tc.tile_pool(name="small", bufs=4))

        pp_ps = ctx.enter_context(tc.tile_pool(name="ppps", bufs=2, space="PSUM"))
        s_ps = ctx.enter_context(tc.tile_pool(name="sps", bufs=4, space="PSUM"))
        av_ps = ctx.enter_context(tc.tile_pool(name="avps", bufs=2, space="PSUM"))

        # ---- constants / broadcasts ----
        # Only the tiny bias tables load before chunk 0's x/weight blocks;
        # masks and LN broadcasts (needed ~40us in) are deferred so the DMA
        # engines serve the first projections immediately.
        bq_sb = singles.tile([128, 8], f32)
        b1_sb = singles.tile([128, 16], f32)
        cm_b2 = singles.tile([WIN, 2, WIN], f16)
        m0_b2 = singles.tile([WIN, 2, WIN], f16)
        g1b = singles.tile([128, D], f16)
        g2b = singles.tile([128, D], f16)
        b2b = singles.tile([128, D], f16)
        b2pb = singles.tile([128, D], f16)

        def load_consts():
            nc.sync.dma_start(
                out=cm_b2,
                in_=bass.AP(tensor=cmkd.tensor, offset=cmkd.offset,
                            ap=[list(cmkd.ap[0]), [0, 2], list(cmkd.ap[1])]),
            )
            nc.sync.dma_start(
                out=m0_b2,
                in_=bass.AP(tensor=m0d.tensor, offset=m0d.offset,
                            ap=[list(m0d.ap[0]), [0, 2], list(m0d.ap[1])]),
            )
            nc.gpsimd.dma_start(out=g1b, in_=bcast_ap(g1d))
            nc.gpsimd.dma_start(out=g2b, in_=bcast_ap(g2d))
            nc.gpsimd.dma_start(out=b2b, in_=bcast_ap(bt2d))
            # ff2 bias (+ folded ln1_b) broadcast: added to the xhb residual
            # on the DVE instead of costing a 512-free PE matmul per FF2 tile
            nc.gpsimd.dma_start(out=b2pb, in_=bcast_ap(b2d[0]))

        epst = singles.tile([128, 1], f32)
        nc.vector.memset(epst, LN_EPS)
        ebt = singles.tile([128, 1], f32)
        nc.vector.memset(ebt, EBT)

        # ---- resident W1 / W2 (loaded once; DMAs emitted after chunk 0's
        # projection loads so they don't delay the first Q/K/V blocks) ----
        w1s = w12_pool.tile([128, 8, FH], f16)
        w2s = w12_pool.tile([128, 16, D], f16)

        def w12_piece(k):
            # W1/W2 resident pieces ride the sync queue AFTER chunk 0's
            # projection loads (hard in-order), interleaved into the window
            # flow so xp loads are not starved.
            if k < 4:
                nc.sync.dma_start(
                    out=w1s[:, :, k * 512:(k + 1) * 512],
                    in_=w1_r[:, :, k * 512:(k + 1) * 512],
                )
            else:
                k -= 4
                nc.sync.dma_start(
                    out=w2s[:, :, k * 256:(k + 1) * 256],
                    in_=w2_r[:, :, k * 256:(k + 1) * 256],
                )

        # rings of per-window K/V tiles (index = global window, -1 = halo)
        ktiles = {}
        vtiles = {}

        def proj_q_block(bi, xt_t, qt_t, wqb=None, split=False):
            if wqb is None:
                wqb = wblk_pool.tile([128, 8, 256], f16, tag="wblk",
                                     name="wqb")
                nc.sync.dma_start(
                    out=wqb, in_=wq_r[:, :, bi * 256:(bi + 1) * 256])
            for sub in range(2):
                qc = bi * 2 + sub
                ps = pp_ps.tile([128, 512], f32, tag="pp")
                if split:
                    # half-token accumulation groups so the cold-start
                    # matmuls only need the first half-DMAs
                    for th in range(2):
                        for d in range(8):
                            nc.tensor.matmul(
                                ps[:, th * 256:(th + 1) * 256],
                                lhsT=wqb[:, d, sub * 128:(sub + 1) * 128],
                                rhs=xt_t[:, d, th * 256:(th + 1) * 256],
                                start=(d == 0), stop=(d == 7),
                            )
                else:
                    for d in range(8):
                        nc.tensor.matmul(
                            ps, lhsT=wqb[:, d, sub * 128:(sub + 1) * 128],
                            rhs=xt_t[:, d, :],
                            start=(d == 0), stop=(d == 7),
                        )
                if qc < 4:
                    nc.scalar.activation(
                        qt_t[:, qc, :], ps, AF.Identity,
                        bias=bq_sb[:, qc:qc + 1], scale=1.0,
                    )
                else:
                    # late qc blocks drain via DVE so the ACT queue is clear
                    # for the first exps at the P->A boundary
                    nc.vector.tensor_scalar(
                        qt_t[:, qc, :], ps, bq_sb[:, qc:qc + 1], None,
                        op0=ALU.add,
                    )

        def proj_k_block(bi, xt_t, xth_t, chn):
            """K projection for weight block bi (256 feature cols)."""
            wkb = wblk_pool.tile([128, 8, 256], f16, tag="wblk")
            nc.sync.dma_start(out=wkb, in_=wk_r[:, :, bi * 256:(bi + 1) * 256])
            for sub in range(2):
                kc = bi * 2 + sub
                ps = pp_ps.tile([128, 512], f32, tag="pp")
                for d in range(8):
                    nc.tensor.matmul(
                        ps, lhsT=wkb[:, d, sub * 128:(sub + 1) * 128],
                        rhs=xt_t[:, d, :],
                        start=(d == 0), stop=(d == 7),
                    )
                for w in range(CW):
                    nc.vector.tensor_copy(
                        out=ktiles[chn * CW + w][:, kc, :],
                        in_=ps[:, w * 128:(w + 1) * 128],
                    )
                if xth_t is not None:
                    ps2 = pp_ps.tile([128, 512], f32, tag="pp")
                    for d in range(8):
                        nc.tensor.matmul(
                            ps2[:, 0:128],
                            lhsT=wkb[:, d, sub * 128:(sub + 1) * 128],
                            rhs=xth_t[:, d, :],
                            start=(d == 0), stop=(d == 7),
                        )
                    nc.vector.tensor_copy(
                        out=ktiles[-1][:, kc, :], in_=ps2[:, 0:128]
                    )

        def proj_v_block(bi, xt_t, xth_t, chn):
            """V projection for weight block bi (256 feature cols = 4 heads)."""
            wvb = wblk_pool.tile([128, 8, 256], f16, tag="wblk")
            nc.sync.dma_start(out=wvb, in_=wv_r[:, :, bi * 256:(bi + 1) * 256])
            for w in range(CW):
                ps = pp_ps.tile([128, 512], f32, tag="pp")
                for d in range(8):
                    nc.tensor.matmul(
                        ps[:, 0:256], lhsT=xt_t[:, d, w * 128:(w + 1) * 128],
                        rhs=wvb[:, d, :],
                        start=(d == 0), stop=(d == 7),
                    )
                nc.vector.tensor_copy(
                    out=vtiles[chn * CW + w][:, bi * 4:(bi + 1) * 4, 0:DH],
                    in_=ps[:, 0:256].rearrange("p (h e) -> p h e", e=DH),
                )
            if xth_t is not None:
                ps2 = pp_ps.tile([128, 512], f32, tag="pp")
                for d in range(8):
                    nc.tensor.matmul(
                        ps2[:, 0:256], lhsT=xth_t[:, d, :], rhs=wvb[:, d, :],
                        start=(d == 0), stop=(d == 7),
                    )
                nc.vector.tensor_copy(
                    out=vtiles[-1][:, bi * 4:(bi + 1) * 4, 0:DH],
                    in_=ps2[:, 0:256].rearrange("p (h e) -> p h e", e=DH),
                )
            return wvb

        def emit_ff1(chn, pair, xht_list, ht_out, hcs=range(16)):
            """FF1 for window pair (256 tokens): ht_out[128, 16, 256]."""
            for hc in hcs:
                ps = pp_ps.tile([128, 512], f32, tag="pp")
                for half in range(2):
                    for d in range(8):
                        nc.tensor.matmul(
                            ps[:, half * 128:(half + 1) * 128],
                            lhsT=w1s[:, d, hc * 128:(hc + 1) * 128],
                            rhs=xht_list[pair * 2 + half][:, d, :],
                            start=(d == 0), stop=(d == 7),
                        )
                nc.scalar.activation(
                    ht_out[:, hc, :], ps[:, 0:256], AF.Relu,
                    bias=b1_sb[:, hc:hc + 1], scale=1.0,
                )

        def emit_ff2_ln2(chn, t, ht_t, tl, xhb_t, p2_t, store_q=None):
            """FF2 + residual + LN2 + store for token tile t (128 tokens)."""
            g = chn * CW + t
            st2 = small.tile([128, 2, 6], f32, tag="st2")
            for yc in range(2):
                ps = pp_ps.tile([128, 512], f32, tag="pp")
                for hc in range(16):
                    nc.tensor.matmul(
                        ps, lhsT=ht_t[:, hc, tl * 128:(tl + 1) * 128],
                        rhs=w2s[:, hc, yc * 512:(yc + 1) * 512],
                        start=(hc == 0), stop=(hc == 15),
                    )
                nc.vector.tensor_add(
                    p2_t[:, yc * 512:(yc + 1) * 512], ps,
                    xhb_t[:, yc * 512:(yc + 1) * 512],
                )
                # stats per half as soon as that half's residual add lands
                nc.vector.bn_stats(
                    out=st2[:, yc, :],
                    in_=p2_t[:, yc * 512:(yc + 1) * 512],
                )
            mv2 = small.tile([128, 2], f32, tag="mv2")
            nc.vector.bn_aggr(out=mv2, in_=st2)
            lv2 = small.tile([128, 1], f32, tag="lv2")
            nc.scalar.activation(lv2, mv2[:, 1:2], AF.Ln, bias=epst)
            rstd2 = small.tile([128, 1], f32, tag="rstd2")
            nc.scalar.activation(rstd2, lv2, AF.Exp, scale=-0.5)
            nmr2 = small.tile([128, 1], f32, tag="nmr2")
            nc.vector.tensor_scalar(
                nmr2, mv2[:, 0:1], rstd2, -1.0, op0=ALU.mult, op1=ALU.mult
            )
            oh = oh_pool.tile([128, D], f16, tag="oh")
            if store_q is not None:
                # program tail: half-granular apply/scale/store so the last
                # transfer starts while the second half is still computing
                for yc in range(2):
                    sl = slice(yc * 512, (yc + 1) * 512)
                    nc.scalar.activation(
                        oh[:, sl], p2_t[:, sl], AF.Identity,
                        bias=nmr2, scale=rstd2,
                    )
                    nc.vector.tensor_mul(oh[:, sl], oh[:, sl], g2b[:, sl])
                    nc.vector.tensor_add(oh[:, sl], oh[:, sl], b2b[:, sl])
                    store_q.dma_start(out=out_r[g][:, sl], in_=oh[:, sl])
            else:
                nc.scalar.activation(
                    oh, p2_t, AF.Identity, bias=nmr2, scale=rstd2)
                nc.vector.tensor_mul(oh, oh, g2b)
                nc.vector.tensor_add(oh, oh, b2b)
                nc.sync.dma_start(out=out_r[g], in_=oh)

        # deferred FF work queue: list of thunks, popped between windows
        prev_tail = None
        hoisted_q = {}
        hoisted_xt = {}
        for chn in range(NCH):
            c0 = HALO + chn * CT

            # ---- x^T slab for this chunk's own 512 tokens ----
            wqb0 = None
            if chn == 0:
                # cold start: interleave half-sized weight/x pieces so the
                # first matmuls can begin after ~2 half-DMAs instead of
                # waiting for both full transfers
                wqb0 = wblk_pool.tile([128, 8, 256], f16, tag="wblk",
                                      name="wqb")
                xt_t = xt_pool.tile([128, 8, CT], f16, name="xt_t")
                nc.sync.dma_start(out=wqb0[:, :, 0:128], in_=wq_r[:, :, 0:128])
                nc.sync.dma_start(out=xt_t[:, :, 0:256],
                                  in_=xT_r[:, :, c0:c0 + 256])
                nc.sync.dma_start(out=wqb0[:, :, 128:256],
                                  in_=wq_r[:, :, 128:256])
                nc.sync.dma_start(out=xt_t[:, :, 256:512],
                                  in_=xT_r[:, :, c0 + 256:c0 + CT])
                # bias tables ride behind the critical cold-start pieces
                nc.sync.dma_start(out=bq_sb, in_=bqd)
                nc.sync.dma_start(out=b1_sb, in_=b1d)
            elif chn in hoisted_xt:
                xt_t = hoisted_xt.pop(chn)
            else:
                xt_t = xt_pool.tile([128, 8, CT], f16, name="xt_t")
                nc.sync.dma_start(out=xt_t, in_=xT_r[:, :, c0:c0 + CT])
            xth_t = None
            if chn == 0:
                xth_t = xt_pool.tile([128, 8, HALO], f16, tag="xth", bufs=1)
                ktiles[-1] = kt_pool.tile([128, 8, WIN], f16, tag="kt", name="kth")
                vtiles[-1] = v_pool.tile([128, H, DH + 1], f16, tag="vt", name="vth")
                nc.vector.memset(vtiles[-1][:, :, DH:DH + 1], 1.0)

            for w in range(CW):
                wg = chn * CW + w
                ktiles[wg] = kt_pool.tile([128, 8, WIN], f16, tag="kt", name="ktw")
                vtiles[wg] = v_pool.tile([128, H, DH + 1], f16, tag="vt", name="vtw")
                nc.vector.memset(vtiles[wg][:, :, DH:DH + 1], 1.0)

            # ---- P(n): projections ----
            if chn in hoisted_q:
                qt_t = hoisted_q.pop(chn)
            else:
                qt_t = qt_pool.tile([128, 8, CT], f16, name="qt_t")
                proj_q_block(0, xt_t, qt_t, wqb0, split=(chn == 0))
                for bi in range(1, 4):
                    proj_q_block(bi, xt_t, qt_t)
            if chn == 0:
                nc.sync.dma_start(out=xth_t, in_=xT_r[:, :, 0:HALO])
                load_consts()
            for bi in range(4):
                proj_k_block(bi, xt_t, xth_t, chn)
            for bi in range(4):
                proj_v_block(bi, xt_t, xth_t, chn)

            # The next chunk's Q projection is hoisted into this chunk's
            # attention bubbles (extra independent PE work right where the
            # exp/mask waits occur, and it shortens the next P phase).  The
            # DMAs are enqueued here, right after this chunk's loads, so the
            # blocks land before the hoisted matmuls run.
            q1_blocks = None
            if chn < NCH - 1:
                nc0 = HALO + (chn + 1) * CT
                xt1 = xt_pool.tile([128, 8, CT], f16, name="xt_t")
                nc.sync.dma_start(out=xt1, in_=xT_r[:, :, nc0:nc0 + CT])
                q1_blocks = []
                for bi in range(4):
                    wqb = wblk_pool.tile([128, 8, 256], f16, tag="wblk",
                                         name="wqb")
                    nc.sync.dma_start(
                        out=wqb, in_=wq_r[:, :, bi * 256:(bi + 1) * 256])
                    q1_blocks.append(wqb)
                qt1 = qt_pool.tile([128, 8, CT], f16, name="qt_t")
                hoisted_xt[chn + 1] = xt1
                hoisted_q[chn + 1] = qt1

            # ---- A(n) + interleaved F work ----
            xhb_list = {}
            xht_list = {}
            ht_pair = {}
            p2_list = {}
            win_state = {}

            def s_phase(w, chn=chn, qt_t=qt_t):
                wg = chn * CW + w
                xpt = xp_pool.tile([128, D], f16, tag="xp")
                nc.sync.dma_start(out=xpt, in_=xp_r[wg])
                es = es_pool.tile([128, H, 2 * WIN], f16, tag="es")
                win_state[w] = (xpt, es)
                # Scores, 4 heads per PSUM group.  Heads are permuted into
                # parity slots (even heads -> slots 0-7 read partitions 0:64,
                # odd heads -> slots 8-15 read 64:128) so every matmul into a
                # given PSUM tile uses the SAME lhsT partition offset — mixing
                # offsets within one PSUM tile is rejected by the hardware.
                # slot s holds head HOS[s]:
                #   HOS[s] = 2*s      for s < 8   (even heads)
                #   HOS[s] = 2*(s-8)+1 for s >= 8 (odd heads)
                # 2-head one-bank PSUM tiles, 4 bufs: twice the S->exp->AV
                # pipeline depth of the 4-head layout.  Emission alternates
                # parity so each rotating buffer only ever sees one lhsT
                # partition offset (hardware constraint).
                for g2 in (0, 4, 1, 5, 2, 6, 3, 7):
                    sps = s_ps.tile([128, 2, 2 * WIN], f32, tag="s")
                    off = 0 if g2 < 4 else 64
                    for j in range(2):
                        s = g2 * 2 + j
                        h = 2 * s if s < 8 else 2 * (s - 8) + 1
                        kprev = ktiles[wg - 1][off:off + 64, h // 2, :]
                        kcur = ktiles[wg][off:off + 64, h // 2, :]
                        qw = qt_t[off:off + 64, h // 2,
                                  w * 128:(w + 1) * 128]
                        nc.tensor.matmul(
                            sps[:, j, 0:128], lhsT=kprev, rhs=qw,
                            start=True, stop=True,
                        )
                        nc.tensor.matmul(
                            sps[:, j, 128:256], lhsT=kcur, rhs=qw,
                            start=True, stop=True,
                        )
                    sl = slice(g2 * 2, (g2 + 1) * 2)
                    nc.scalar.activation(es[:, sl, :], sps, AF.Exp, bias=ebt)
                    # masks on the idle GPSIMD engine (DVE-queue variants
                    # regressed: its queue is deep exactly when AV waits)
                    nc.gpsimd.tensor_mul(
                        es[:, sl, 128:256], es[:, sl, 128:256],
                        cm_b2,
                    )
                    if wg == 0:
                        nc.gpsimd.tensor_mul(
                            es[:, sl, 0:128], es[:, sl, 0:128],
                            m0_b2,
                        )

            def av_phase(w, chn=chn):
                wg = chn * CW + w
                xpt, es = win_state.pop(w)
                # AV + fused normalize/residual (same group order as the
                # score phase so the first AV group awaits the first exp)
                at_t = at_pool.tile([128, D], f16, tag="at")
                for g in (0, 2, 1, 3):
                    av = av_ps.tile([128, 4, 128], f32, tag="av")
                    for j in range(4):
                        s = g * 4 + j
                        h = 2 * s if s < 8 else 2 * (s - 8) + 1
                        nc.tensor.matmul(
                            av[:, j, 0:DH + 1], lhsT=es[:, s, 0:128],
                            rhs=vtiles[wg - 1][:, h, :],
                            start=True, stop=False,
                        )
                        nc.tensor.matmul(
                            av[:, j, 0:DH + 1], lhsT=es[:, s, 128:256],
                            rhs=vtiles[wg][:, h, :],
                            start=False, stop=True,
                        )
                    rden = small.tile([128, 4], f32, tag="rden")
                    nc.vector.reciprocal(rden, av[:, :, DH:DH + 1])
                    for j in range(4):
                        s = g * 4 + j
                        h = 2 * s if s < 8 else 2 * (s - 8) + 1
                        nc.vector.scalar_tensor_tensor(
                            out=at_t[:, h * DH:(h + 1) * DH],
                            in0=av[:, j, 0:DH],
                            scalar=rden[:, j:j + 1],
                            in1=xpt[:, h * DH:(h + 1) * DH],
                            op0=ALU.mult, op1=ALU.add,
                        )
                # LN1 -> xhb' = xhat * g1
                stats = small.tile([128, 2, 6], f32, tag="st1")
                atv = at_t.rearrange("p (a b) -> p a b", b=512)
                for sg in range(2):
                    nc.vector.bn_stats(out=stats[:, sg, :], in_=atv[:, sg, :])
                mv = small.tile([128, 2], f32, tag="mv1")
                nc.vector.bn_aggr(out=mv, in_=stats)
                lv = small.tile([128, 1], f32, tag="lv1")
                nc.scalar.activation(lv, mv[:, 1:2], AF.Ln, bias=epst)
                rstd = small.tile([128, 1], f32, tag="rstd1")
                nc.scalar.activation(rstd, lv, AF.Exp, scale=-0.5)
                nmr = small.tile([128, 1], f32, tag="nmr1")
                nc.vector.tensor_scalar(
                    nmr, mv[:, 0:1], rstd, -1.0, op0=ALU.mult, op1=ALU.mult
                )
                tmp = tmp_pool.tile([128, D], f16, tag="tmp")
                nc.scalar.activation(tmp, at_t, AF.Identity, bias=nmr, scale=rstd)
                # transpose the raw x-hat immediately (W1 carries the g1
                # fold); the g1-scaled copy for the FF2 residual is off the
                # critical path
                xht = xht_pool.tile([128, 8, WIN], f16, tag="xht")
                nc.scalar.dma_start_transpose(xht, tmp)
                xht_list[w] = xht
                xhb = xhb_pool.tile([128, D], f16, tag="xhb")
                nc.vector.tensor_mul(xhb, tmp, g1b)
                nc.vector.tensor_add(xhb, xhb, b2pb)
                xhb_list[w] = xhb

            def make_ff1(pair, chn=chn, xht_list=xht_list, ht_pair=ht_pair,
                         half=None):
                def run():
                    if half in (None, 0):
                        ht_pair[pair] = ht_pool.tile(
                            [128, 16, 2 * WIN], f16, tag="ht", name="htp"
                        )
                    hcs = (range(16) if half is None else
                           range(8) if half == 0 else range(8, 16))
                    emit_ff1(chn, pair, xht_list, ht_pair[pair], hcs)
                return run

            def make_ff2(t, chn=chn, xhb_list=xhb_list, ht_pair=ht_pair,
                         p2_list=p2_list):
                def run():
                    p2_list[t] = p2_pool.tile([128, D], f16, tag="p2",
                                              name="p2t")
                    # last chunk's stores ride the fast HWDGE queue so the
                    # program tail isn't the slow SWDGE descriptor pass
                    sq = nc.scalar if (chn == NCH - 1 and t >= 2) else None
                    emit_ff2_ln2(chn, t, ht_pair[t // 2], t % 2,
                                 xhb_list[t], p2_list[t], store_q=sq)
                return run

            # Window/FF interleave with cross-chunk tails: the previous
            # chunk's FF1(pair1)/FF2(t2)/FF2(t3) fill the P->A handoff and
            # the exp-wait bubbles of the first two windows; FF1(pair0)
            # fills window 2's exp wait.
            if prev_tail:
                prev_tail[0]()
            s_phase(0)
            if chn == 0:
                for k in range(4):
                    w12_piece(k)
            if prev_tail:
                prev_tail[1]()
            if q1_blocks:
                proj_q_block(0, xt1, qt1, q1_blocks[0])
                proj_q_block(1, xt1, qt1, q1_blocks[1])
            av_phase(0)
            s_phase(1)
            if chn == 0:
                for k in range(4, 8):
                    w12_piece(k)
            if prev_tail:
                prev_tail[2]()
                prev_tail[3]()
            if q1_blocks:
                proj_q_block(2, xt1, qt1, q1_blocks[2])
                proj_q_block(3, xt1, qt1, q1_blocks[3])
            av_phase(1)
            s_phase(2)
            make_ff1(0, half=0)()
            av_phase(2)
            make_ff1(0, half=1)()
            s_phase(3)
            make_ff2(0)()
            av_phase(3)
            make_ff2(1)()
            prev_tail = [make_ff1(1, half=0), make_ff1(1, half=1),
                         make_ff2(2), make_ff2(3)]
            if chn == NCH - 1:
                for fn_ in prev_tail:
                    fn_()
                prev_tail = None

    bacc.get_activation_tables = _pinned_gat
    try:
        nc.compile()
    finally:
        bacc.get_activation_tables = _orig_gat
    return nc


def _get_program():
    if "nc" not in _PROGRAM_CACHE:
        _PROGRAM_CACHE["nc"] = _build_program()
    return _PROGRAM_CACHE["nc"]


def make_in_maps(x, q_proj, k_proj, v_proj, q_bias, k_bias, v_bias,
                 ln1_g, ln1_b, ln2_g, ln2_b, ff1_w, ff1_b, ff2_w, ff2_b):
    """Host-side prep: fold biases/scales, shard across 8 cores."""
    x = np.asarray(x, np.float32)
    scale = DH ** -0.5

    Wq = (np.transpose(np.asarray(q_proj, np.float32), (1, 0, 2)).reshape(D, D)
          * scale).astype(np.float16)
    Wk = np.transpose(np.asarray(k_proj, np.float32), (1, 0, 2)).reshape(D, D).astype(np.float16)
    Wv = np.transpose(np.asarray(v_proj, np.float32), (1, 0, 2)).reshape(D, D).astype(np.float16)
    bq_full = (np.asarray(q_bias, np.float32).reshape(D) * scale)
    bv_full = np.asarray(v_bias, np.float32).reshape(D)

    ln1_g = np.asarray(ln1_g, np.float32)
    ln1_b = np.asarray(ln1_b, np.float32)
    ff1_w = np.asarray(ff1_w, np.float32)
    ff1_b = np.asarray(ff1_b, np.float32)
    ff2_w = np.asarray(ff2_w, np.float32)
    ff2_b = np.asarray(ff2_b, np.float32)

    W1 = (ff1_w * ln1_g[:, None]).astype(np.float16)  # fold ln1_g into rows
    b1_full = ff1_b + ln1_b @ ff1_w                   # fold ln1_b into ff1 bias
    W2 = ff2_w.astype(np.float16)
    b2_full = (ff2_b + ln1_b).astype(np.float16)      # fold ln1_b into ff2 bias

    bq_l = np.ascontiguousarray(bq_full.reshape(8, 128).T.astype(np.float32))
    b1_l = np.ascontiguousarray(b1_full.reshape(16, 128).T.astype(np.float32))

    kq = np.arange(WIN)
    # multiplicative causal mask, [k, q]: 1 where q >= k else 0
    cmk = (kq[None, :] >= kq[:, None]).astype(np.float16)

    common = {
        "wq": Wq, "wk": Wk, "wv": Wv,
        "bq": bq_l, "w1": W1, "b1": b1_l, "w2": W2,
        "b2": b2_full.reshape(1, D),
        "g1": ln1_g.astype(np.float16),
        "g2": np.asarray(ln2_g, np.float16),
        "bt2": np.asarray(ln2_b, np.float16),
        "cmk": cmk,
    }

    in_maps = []
    for b in range(B):
        for s in range(2):
            own = x[b, s * T:(s + 1) * T]
            if s == 0:
                halo = np.zeros((HALO, D), np.float32)
                m0 = np.zeros((WIN, WIN), np.float16)
            else:
                halo = x[b, s * T - HALO:s * T]
                m0 = np.ones((WIN, WIN), np.float16)
            xta = np.ascontiguousarray(
                np.concatenate([halo, own], axis=0).T).astype(np.float16)
            xpa = (own + bv_full[None, :]).astype(np.float16)
            in_maps.append({
                **common,
                "xT": xta,
                "xp": np.ascontiguousarray(xpa),
                "m0": m0,
            })
    return in_maps


def gather_outputs(results):
    out = np.empty((B, S, D), np.float32)
    for b in range(B):
        for s in range(2):
            out[b, s * T:(s + 1) * T] = results[b * 2 + s]["out"].astype(np.float32)
    return out


def kernel(**inputs):
    from concourse import bass_utils

    nc = _get_program()
    in_maps = make_in_maps(**inputs)
    res = bass_utils.run_bass_kernel_spmd(nc, in_maps, core_ids=list(range(8)))
    return gather_outputs(res.results)
